# revision 6
# baseline (speedup 1.0000x reference)
"""BoT block v3: fp8 DR GEMMs, 2-bank PSUM tiles, PE-side softmax denom,
interleaved cross-image emission to keep all engines fed.

Pipeline: A(0) | B(0)+A(1) | C(0)+B(1) | C(1)
  A = conv1+BN+ReLU -> feat; qk -> q,k; v
  B = per head: logits -> exp(et) -> denB (PE DR reduce+bcast) -> rc1
      (Newton) -> attn -> atf = relu(attn)*rc1
  C = conv3 + residual + BN + ReLU -> out (stored as 64x, host /64)

Scales: w1*64 (bn1 /64); q = 16*q (qk_w q*SCALE*64, evict *.25); k = 8*k
(qk_w k*8, +8*embT); v = 8*v (v_w*8); logits psum = 128*l (exp scale 1/128);
denB = -0.0625*denom (DR all-const lhsT); rc1 = denB*r0^2/0.0625 + 2*r0;
atf = 8*relu(attn); w3*inv3*8 -> psum 64x; out stored 64x f16.
"""
import numpy as np
import ml_dtypes

import concourse.bass as bass
import concourse.mybir as mybir
import concourse.tile as tile
from concourse import bacc
from concourse.masks import make_identity
from concourse.bass_utils import run_bass_kernel_spmd

EPS = 1e-5
HEADS = 4
DQK = 128
DV = 128
SCALE = DQK ** -0.5
N_IMG = 16
CIN = 2048
H = W = 32
HW = H * W
MID = HEADS * DV
NCORES = 8
IMGS = N_IMG // NCORES

P = 128
F16 = mybir.dt.float16
F32 = mybir.dt.float32
F8 = mybir.dt.float8e4
AF = mybir.ActivationFunctionType
ALU = mybir.AluOpType
DR = mybir.MatmulPerfMode.DoubleRow

K2_1 = CIN // 256
K2_2 = MID // 256
OT3 = CIN // P
YT = HW // P

SEED = 0.0625                  # e4m3-exact |lhsT| value for denom reduce
R0 = SEED / 64.0               # Newton seed ~ 1/1024-center
RC_A = R0 * R0 / SEED          # rc1 = RC_A*denB + RC_B
RC_B = 2.0 * R0

OUT_DIV = 64.0
_BUILT = {}


def _build():
    if "nc" in _BUILT:
        return _BUILT["nc"]
    nc = bacc.Bacc("TRN2", target_bir_lowering=False, debug=False,
                   num_devices=NCORES)

    x8_d = nc.dram_tensor("x8", [IMGS, K2_1, P, 2, HW], F8, kind="ExternalInput")
    xres_d = nc.dram_tensor("xres", [IMGS, OT3, P, HW], F16, kind="ExternalInput")
    w1t_d = nc.dram_tensor("w1t", [K2_1, P, 2, MID], F8, kind="ExternalInput")
    qkwt_d = nc.dram_tensor("qkwt", [K2_2, P, 2, 2 * MID], F8, kind="ExternalInput")
    vwt_d = nc.dram_tensor("vwt", [K2_2, P, 2, MID], F8, kind="ExternalInput")
    w3t_d = nc.dram_tensor("w3t", [K2_2, P, 2, CIN], F8, kind="ExternalInput")
    embt_d = nc.dram_tensor("embt", [P, HW], F32, kind="ExternalInput")
    bn1_d = nc.dram_tensor("bn1", [P, 2, MID // P], F32, kind="ExternalInput")
    add3_d = nc.dram_tensor("add3", [P, OT3], F32, kind="ExternalInput")
    out_d = nc.dram_tensor("out", [IMGS, OT3, P, HW], F16, kind="ExternalOutput")

    with tile.TileContext(nc) as tc:
        with (
            tc.tile_pool(name="consts", bufs=1) as consts,
            tc.tile_pool(name="xpool", bufs=10) as xpool,
            tc.tile_pool(name="feat", bufs=2) as featp,
            tc.tile_pool(name="qk", bufs=2) as qkp,
            tc.tile_pool(name="vp", bufs=2) as vp,
            tc.tile_pool(name="et", bufs=3) as etp,
            tc.tile_pool(name="rc", bufs=2) as rcp,
            tc.tile_pool(name="atf", bufs=2) as atfp,
            tc.tile_pool(name="xres", bufs=18) as xresp,
            tc.tile_pool(name="outp", bufs=4) as outp,
            tc.tile_pool(name="ps_mm", bufs=2, space="PSUM") as ps_mm,
            tc.tile_pool(name="ps_l", bufs=2, space="PSUM") as ps_l,
        ):
            w1t = consts.tile([P, K2_1, 2, MID], F8)
            bn1 = consts.tile([P, 2, MID // P], F32)
            qkwt = consts.tile([P, K2_2, 2, 2 * MID], F8)
            vwt = consts.tile([P, K2_2, 2, MID], F8)
            w3t = consts.tile([P, K2_2, 2, CIN], F8)
            embt = consts.tile([P, HW], F32)
            add3 = consts.tile([P, OT3], F32)
            ones8 = consts.tile([P, 2, P], F8)
            nc.vector.memset(ones8[:], -SEED)
            ident = consts.tile([P, P], F16)
            make_identity(nc, ident[:])

            nc.sync.dma_start(bn1[:], bn1_d.ap())

            def load_deferred_consts():
                for k in range(K2_2):
                    nc.sync.dma_start(qkwt[:, k], qkwt_d.ap()[k])
                for k in range(K2_2):
                    nc.sync.dma_start(vwt[:, k], vwt_d.ap()[k])
                for k in range(K2_2):
                    nc.sync.dma_start(w3t[:, k], w3t_d.ap()[k])
                nc.sync.dma_start(embt[:], embt_d.ap())
                nc.sync.dma_start(add3[:], add3_d.ap())

            # ---------- phase A pieces ----------
            def load_x(i):
                # x on the scalar hwdge ring, weights on sync: two DMA rings
                # stream in parallel so conv1(0) is not DMA-paced
                x_tiles = []
                for k in range(K2_1):
                    if i == 0:
                        nc.sync.dma_start(w1t[:, k], w1t_d.ap()[k])
                    x_sb = xpool.tile([P, 2, HW], F8, tag="x", name=f"x_{i}_{k}")
                    (nc.scalar if i == 0 else nc.sync).dma_start(
                        x_sb[:], x8_d.ap()[i, k])
                    x_tiles.append(x_sb)
                    if i == 0 and k == K2_1 - 1:
                        load_deferred_consts()
                return x_tiles

            def conv1_chunk(i, ot, x_tiles, feat, st, j):
                if j == 0:
                    st["ps"] = ps_mm.tile([P, HW], F32, tag="mm",
                                          name=f"psc1_{i}_{ot}")
                ps = st["ps"]
                for k in range(2 * j, 2 * j + 2):
                    for nh in range(2):
                        nc.tensor.matmul(
                            ps[:, nh * 512:(nh + 1) * 512],
                            w1t[:, k, :, ot * P:(ot + 1) * P],
                            x_tiles[k][:, :, nh * 512:(nh + 1) * 512],
                            start=(k == 0), stop=(k == K2_1 - 1),
                            perf_mode=DR,
                        )
                if j == K2_1 // 2 - 1:
                    nc.scalar.activation(
                        feat[:, ot // 2, ot % 2], ps[:], AF.Relu,
                        scale=bn1[:, 0, ot:ot + 1], bias=bn1[:, 1, ot:ot + 1])

            def conv1_ot(i, ot, x_tiles, feat):
                st = {}
                for j in range(K2_1 // 2):
                    conv1_chunk(i, ot, x_tiles, feat, st, j)

            def conv1_fillers(i, ot, x_tiles, feat):
                st = {}
                return [
                    (lambda j=j: conv1_chunk(i, ot, x_tiles, feat, st, j))
                    for j in range(K2_1 // 2)
                ]

            def qk_ot(i, ot, feat, q_sb, k_sb):
                ps = ps_mm.tile([P, HW], F32, tag="mm", name=f"psqk_{i}_{ot}")
                for k in range(K2_2):
                    for nh in range(2):
                        nc.tensor.matmul(
                            ps[:, nh * 512:(nh + 1) * 512],
                            qkwt[:, k, :, ot * P:(ot + 1) * P],
                            feat[:, k, :, nh * 512:(nh + 1) * 512],
                            start=(k == 0), stop=(k == K2_2 - 1),
                            perf_mode=DR,
                        )
                if ot < HEADS:
                    nc.scalar.activation(q_sb[:, ot], ps[:], AF.Copy, scale=0.25)
                else:
                    nc.vector.tensor_tensor(k_sb[:, ot - HEADS], ps[:],
                                            embt[:], ALU.add)

            def v_pair(i, t, feat, v_sb):
                ps = ps_mm.tile([P, HW], F32, tag="mm", name=f"psv_{i}_{t}")
                for half in range(2):
                    yt = 2 * t + half
                    for k in range(K2_2):
                        nc.tensor.matmul(
                            ps[:, half * 512:(half + 1) * 512],
                            feat[:, k, :, yt * P:(yt + 1) * P],
                            vwt[:, k],
                            start=(k == 0), stop=(k == K2_2 - 1),
                            perf_mode=DR,
                        )
                nc.vector.tensor_copy(
                    v_sb[:, 2 * t:2 * t + 2],
                    ps[:].rearrange("p (t h d) -> p t h d", t=2, d=DV))

            # ---------- phase B (one head) ----------
            def head_logits(i, h, q_sb, k_sb, et, fillers=()):
                fillers = list(fillers)
                for yj in range(YT):
                    psl = ps_l.tile([P, HW], F32, tag="l", name=f"psl_{i}_{h}_{yj}")
                    for nh in range(2):
                        nc.tensor.matmul(
                            psl[:, nh * 512:(nh + 1) * 512],
                            k_sb[:, h, yj * P:(yj + 1) * P],
                            q_sb[:, h, nh * 512:(nh + 1) * 512],
                            start=True, stop=True,
                        )
                    nc.scalar.activation(et[:, yj], psl[:], AF.Exp,
                                         scale=1.0 / 128.0)
                    if yj % 2 == 1 and fillers:
                        fillers.pop(0)()
                for f in fillers:
                    f()

            def head_attn(i, h, et, v_sb, atf):
                dnb = ps_mm.tile([P, HW], F32, tag="mm", name=f"dnb_{i}_{h}")
                for nh in range(2):
                    for a in range(YT // 2):
                        nc.tensor.matmul(
                            dnb[:, nh * 512:(nh + 1) * 512],
                            ones8[:],
                            et[:, 2 * a:2 * a + 2, nh * 512:(nh + 1) * 512],
                            start=(a == 0), stop=(a == YT // 2 - 1),
                            perf_mode=DR,
                        )
                rc1 = rcp.tile([P, HW], F16, tag="rc", name=f"rc_{i}_{h}")
                nc.vector.tensor_scalar(rc1[:], dnb[:], RC_A, RC_B,
                                        ALU.mult, ALU.add)
                psa = ps_l.tile([P, HW], F32, tag="l", name=f"psa_{i}_{h}")
                for nh in range(2):
                    for a in range(YT // 2):
                        nc.tensor.matmul(
                            psa[:, nh * 512:(nh + 1) * 512],
                            v_sb[:, 2 * a:2 * a + 2, h],
                            et[:, 2 * a:2 * a + 2, nh * 512:(nh + 1) * 512],
                            start=(a == 0), stop=(a == YT // 2 - 1),
                            perf_mode=DR,
                        )
                nc.vector.scalar_tensor_tensor(atf[:, h], psa[:], 0.0, rc1[:],
                                               ALU.max, ALU.mult)

            # ---------- phase C (one out-tile) ----------
            def conv3_ot(i, ot, atf, xres_tiles, evict_dve, resid_pe,
                         alt_pool=False):
                ps = (ps_l.tile([P, HW], F32, tag="l", name=f"psc3_{i}_{ot}")
                      if alt_pool else
                      ps_mm.tile([P, HW], F32, tag="mm",
                                 name=f"psc3_{i}_{ot}"))
                for nh in range(2):
                    sl = slice(nh * 512, (nh + 1) * 512)
                    for k in range(K2_2):
                        nc.tensor.matmul(
                            ps[:, sl],
                            w3t[:, k, :, ot * P:(ot + 1) * P],
                            atf[:, 2 * k:2 * k + 2, sl],
                            start=(k == 0),
                            stop=(not resid_pe and k == K2_2 - 1),
                            perf_mode=DR,
                        )
                    if resid_pe:
                        nc.tensor.matmul(ps[:, sl], ident[:],
                                         xres_tiles[ot][:, sl],
                                         start=False, stop=True)
                if not resid_pe:
                    nc.vector.tensor_tensor(ps[:], ps[:], xres_tiles[ot][:],
                                            ALU.add)
                o_sb = outp.tile([P, HW], F16, tag="o", name=f"o_{i}_{ot}")
                if evict_dve:
                    nc.vector.tensor_scalar(o_sb[:], ps[:], add3[:, ot:ot + 1],
                                            0.0, ALU.add, ALU.max)
                else:
                    nc.scalar.activation(o_sb[:], ps[:], AF.Relu,
                                         bias=add3[:, ot:ot + 1])
                nc.scalar.dma_start(out_d.ap()[i, ot], o_sb[:])

            def prefetch_xres(i):
                tiles = []
                for ot in range(OT3):
                    xr = xresp.tile([P, HW], F16, tag="xr", name=f"xr_{i}_{ot}")
                    nc.sync.dma_start(xr[:], xres_d.ap()[i, ot])
                    tiles.append(xr)
                return tiles

            def alloc_bqv(i):
                feat = featp.tile([P, K2_2, 2, HW], F8, tag="feat",
                                  name=f"feat_{i}")
                q_sb = qkp.tile([P, HEADS, HW], F8, tag="q", name=f"q_{i}")
                k_sb = qkp.tile([P, HEADS, HW], F8, tag="k", name=f"k_{i}")
                v_sb = vp.tile([P, YT, HEADS, DV], F8, tag="v", name=f"v_{i}")
                return feat, q_sb, k_sb, v_sb

            # ================= emission =================
            # A(0)
            feat0, q0, k0, v0 = alloc_bqv(0)
            with nc.named_scope("A0"):
                xt0 = load_x(0)
                for ot in range(MID // P):
                    conv1_ot(0, ot, xt0, feat0)
                for ot in range(2 * HEADS):
                    qk_ot(0, ot, feat0, q0, k0)
                for t in range(YT // 2):
                    v_pair(0, t, feat0, v0)
            xt1 = load_x(1)
            xres0 = prefetch_xres(0)

            # B(0) interleaved with A(1)
            feat1, q1, k1, v1 = alloc_bqv(1)
            atf0 = atfp.tile([P, HEADS, HW], F8, tag="atf", name="atf_0")
            with nc.named_scope("B0_A1"):
                for h in range(HEADS):
                    et = etp.tile([P, YT, HW], F8, tag="et", name=f"et_0_{h}")
                    head_logits(0, h, q0, k0, et,
                                conv1_fillers(1, h, xt1, feat1))
                    head_attn(0, h, et, v0, atf0)
                for ot in range(2 * HEADS):
                    qk_ot(1, ot, feat1, q1, k1)
                for t in range(YT // 2):
                    v_pair(1, t, feat1, v1)
            xres1 = prefetch_xres(1)

            # C(0) interleaved with B(1)
            atf1 = atfp.tile([P, HEADS, HW], F8, tag="atf", name="atf_1")
            with nc.named_scope("C0_B1"):
                for h in range(HEADS):
                    et = etp.tile([P, YT, HW], F8, tag="et", name=f"et_1_{h}")
                    head_logits(1, h, q1, k1, et, [
                        (lambda ot=4 * h + j: conv3_ot(
                            0, ot, atf0, xres0, evict_dve=(ot % 2 == 1),
                            resid_pe=False))
                        for j in range(2)
                    ])
                    head_attn(1, h, et, v1, atf1)
                    for j in range(2, 4):
                        ot = 4 * h + j
                        conv3_ot(0, ot, atf0, xres0, evict_dve=(ot % 2 == 1),
                                 resid_pe=False)

            # C(1)
            with nc.named_scope("C1"):
                for ot in range(OT3):
                    conv3_ot(1, ot, atf1, xres1, evict_dve=(ot % 2 == 1),
                             resid_pe=True, alt_pool=(ot % 2 == 1))

    nc.compile()
    _BUILT["nc"] = nc
    return nc


def _prep_maps(x, conv1_w, gamma1, beta1, mean1, var1, qk_w, v_w, pos_h, pos_w,
               conv3_w, gamma3, beta3, mean3, var3):
    f16 = np.float16
    f8 = ml_dtypes.float8_e4m3
    inv1 = (gamma1 / np.sqrt(var1 + EPS)).astype(np.float32)
    add1 = (beta1 - mean1 * inv1).astype(np.float32)
    inv3 = (gamma3 / np.sqrt(var3 + EPS)).astype(np.float32)
    add3 = (beta3 - mean3 * inv3).astype(np.float32)

    def dr_weight(wt, k2):
        c, o = wt.shape
        return np.ascontiguousarray(
            wt.reshape(k2, 2, P, o).transpose(0, 2, 1, 3)).astype(f8)

    w1t = dr_weight(np.ascontiguousarray(conv1_w.T) * 64.0, K2_1)
    qk_mod = np.concatenate([qk_w[:HEADS * DQK] * (SCALE * 64.0),
                             qk_w[HEADS * DQK:] * 8.0], 0)
    qkwt = dr_weight(np.ascontiguousarray(qk_mod.T), K2_2)
    vwt = dr_weight(np.ascontiguousarray(v_w.T) * 8.0, K2_2)
    w3t = dr_weight(np.ascontiguousarray((conv3_w * inv3[:, None]).T) * 8.0,
                    K2_2)
    embt = np.ascontiguousarray(
        (pos_h[:, None, :] + pos_w[None, :, :]).reshape(HW, DQK).T * 8.0
    ).astype(np.float32)
    bn1 = np.stack([(inv1 / 64.0).reshape(MID // P, P).T,
                    add1.reshape(MID // P, P).T], 1)
    bn1 = np.ascontiguousarray(bn1).astype(np.float32)
    # out stored as 64x: bias folded as 64*add3
    add3_t = np.ascontiguousarray(
        (64.0 * add3).reshape(OT3, P).T).astype(np.float32)

    xr = x.reshape(N_IMG, CIN, HW)
    x8_all = np.ascontiguousarray(
        xr.reshape(N_IMG, K2_1, 2, P, HW).transpose(0, 1, 3, 2, 4)).astype(f8)
    xres_all = (xr.reshape(N_IMG, OT3, P, HW) * 64.0).astype(f16)

    in_maps = []
    for c in range(NCORES):
        sl = slice(c * IMGS, (c + 1) * IMGS)
        in_maps.append({
            "x8": np.ascontiguousarray(x8_all[sl]),
            "xres": np.ascontiguousarray(xres_all[sl]),
            "w1t": w1t, "qkwt": qkwt, "vwt": vwt, "w3t": w3t,
            "embt": embt, "bn1": bn1, "add3": add3_t,
        })
    return in_maps


def kernel(**inputs):
    nc = _build()
    inputs = {k: np.asarray(v) for k, v in inputs.items()}
    in_maps = _prep_maps(**inputs)
    res = run_bass_kernel_spmd(nc, in_maps, core_ids=list(range(NCORES)))
    out = np.concatenate([r["out"] for r in res.results], 0)
    return (out.reshape(N_IMG, CIN, H, W).astype(np.float32) / 64.0)


# revision 7
# speedup vs baseline: 1.0046x; 1.0046x over previous
"""BoT block v3: fp8 DR GEMMs, 2-bank PSUM tiles, PE-side softmax denom,
interleaved cross-image emission to keep all engines fed.

Pipeline: A(0) | B(0)+A(1) | C(0)+B(1) | C(1)
  A = conv1+BN+ReLU -> feat; qk -> q,k; v
  B = per head: logits -> exp(et) -> denB (PE DR reduce+bcast) -> rc1
      (Newton) -> attn -> atf = relu(attn)*rc1
  C = conv3 + residual + BN + ReLU -> out (stored as 64x, host /64)

Scales: w1*64 (bn1 /64); q = 16*q (qk_w q*SCALE*64, evict *.25); k = 8*k
(qk_w k*8, +8*embT); v = 8*v (v_w*8); logits psum = 128*l (exp scale 1/128);
denB = -0.0625*denom (DR all-const lhsT); rc1 = denB*r0^2/0.0625 + 2*r0;
atf = 8*relu(attn); w3*inv3*8 -> psum 64x; out stored 64x f16.
"""
import numpy as np
import ml_dtypes

import concourse.bass as bass
import concourse.mybir as mybir
import concourse.tile as tile
from concourse import bacc
from concourse.masks import make_identity
from concourse.bass_utils import run_bass_kernel_spmd

EPS = 1e-5
HEADS = 4
DQK = 128
DV = 128
SCALE = DQK ** -0.5
N_IMG = 16
CIN = 2048
H = W = 32
HW = H * W
MID = HEADS * DV
NCORES = 8
IMGS = N_IMG // NCORES

P = 128
F16 = mybir.dt.float16
F32 = mybir.dt.float32
F8 = mybir.dt.float8e4
AF = mybir.ActivationFunctionType
ALU = mybir.AluOpType
DR = mybir.MatmulPerfMode.DoubleRow

K2_1 = CIN // 256
K2_2 = MID // 256
OT3 = CIN // P
YT = HW // P

SEED = 0.0625                  # e4m3-exact |lhsT| value for denom reduce
R0 = SEED / 64.0               # Newton seed ~ 1/1024-center
RC_A = R0 * R0 / SEED          # rc1 = RC_A*denB + RC_B
RC_B = 2.0 * R0

OUT_DIV = 64.0
_BUILT = {}


def _build():
    if "nc" in _BUILT:
        return _BUILT["nc"]
    nc = bacc.Bacc("TRN2", target_bir_lowering=False, debug=False,
                   num_devices=NCORES)

    x8_d = nc.dram_tensor("x8", [IMGS, K2_1, P, 2, HW], F8, kind="ExternalInput")
    xres_d = nc.dram_tensor("xres", [IMGS, OT3, P, HW], F16, kind="ExternalInput")
    w1t_d = nc.dram_tensor("w1t", [K2_1, P, 2, MID], F8, kind="ExternalInput")
    qkwt_d = nc.dram_tensor("qkwt", [K2_2, P, 2, 2 * MID], F8, kind="ExternalInput")
    vwt_d = nc.dram_tensor("vwt", [K2_2, P, 2, MID], F8, kind="ExternalInput")
    w3t_d = nc.dram_tensor("w3t", [K2_2, P, 2, CIN], F8, kind="ExternalInput")
    embt_d = nc.dram_tensor("embt", [P, HW], F32, kind="ExternalInput")
    bn1_d = nc.dram_tensor("bn1", [P, 2, MID // P], F32, kind="ExternalInput")
    add3_d = nc.dram_tensor("add3", [P, OT3], F32, kind="ExternalInput")
    out_d = nc.dram_tensor("out", [IMGS, OT3, P, HW], F16, kind="ExternalOutput")

    with tile.TileContext(nc) as tc:
        with (
            tc.tile_pool(name="consts", bufs=1) as consts,
            tc.tile_pool(name="xpool", bufs=10) as xpool,
            tc.tile_pool(name="feat", bufs=2) as featp,
            tc.tile_pool(name="qk", bufs=2) as qkp,
            tc.tile_pool(name="vp", bufs=2) as vp,
            tc.tile_pool(name="et", bufs=3) as etp,
            tc.tile_pool(name="rc", bufs=2) as rcp,
            tc.tile_pool(name="atf", bufs=2) as atfp,
            tc.tile_pool(name="xres", bufs=18) as xresp,
            tc.tile_pool(name="outp", bufs=4) as outp,
            tc.tile_pool(name="ps_mm", bufs=2, space="PSUM") as ps_mm,
            tc.tile_pool(name="ps_l", bufs=2, space="PSUM") as ps_l,
        ):
            w1t = consts.tile([P, K2_1, 2, MID], F8)
            bn1 = consts.tile([P, 2, MID // P], F32)
            qkwt = consts.tile([P, K2_2, 2, 2 * MID], F8)
            vwt = consts.tile([P, K2_2, 2, MID], F8)
            w3t = consts.tile([P, K2_2, 2, CIN], F8)
            embt = consts.tile([P, HW], F32)
            add3 = consts.tile([P, OT3], F32)
            ones8 = consts.tile([P, 2, P], F8)
            nc.vector.memset(ones8[:], -SEED)
            ident = consts.tile([P, P], F16)
            make_identity(nc, ident[:])
            # PE warm-up: keep the HAM activity window fed during the
            # initial DMA fill so conv1 starts at 2.4 GHz (the free-running
            # throttle window otherwise re-gates during the ~9us idle).
            wsc = consts.tile([P, 512], F16)
            nc.vector.memset(wsc[:], 0.0)
            wps = ps_mm.tile([P, HW], F32, tag="mm", name="warmup_ps")
            for wi in range(36):
                nc.tensor.matmul(wps[:, 0:512], ident[:], wsc[:],
                                 start=True, stop=True)

            nc.sync.dma_start(bn1[:], bn1_d.ap())

            def load_deferred_consts():
                for k in range(K2_2):
                    nc.sync.dma_start(qkwt[:, k], qkwt_d.ap()[k])
                for k in range(K2_2):
                    nc.sync.dma_start(vwt[:, k], vwt_d.ap()[k])
                for k in range(K2_2):
                    nc.sync.dma_start(w3t[:, k], w3t_d.ap()[k])
                nc.sync.dma_start(embt[:], embt_d.ap())
                nc.sync.dma_start(add3[:], add3_d.ap())

            # ---------- phase A pieces ----------
            def load_x(i):
                # x on the scalar hwdge ring, weights on sync: two DMA rings
                # stream in parallel so conv1(0) is not DMA-paced
                x_tiles = []
                for k in range(K2_1):
                    if i == 0:
                        nc.sync.dma_start(w1t[:, k], w1t_d.ap()[k])
                    x_sb = xpool.tile([P, 2, HW], F8, tag="x", name=f"x_{i}_{k}")
                    (nc.scalar if i == 0 else nc.sync).dma_start(
                        x_sb[:], x8_d.ap()[i, k])
                    x_tiles.append(x_sb)
                    if i == 0 and k == K2_1 - 1:
                        load_deferred_consts()
                return x_tiles

            def conv1_chunk(i, ot, x_tiles, feat, st, j):
                if j == 0:
                    st["ps"] = ps_mm.tile([P, HW], F32, tag="mm",
                                          name=f"psc1_{i}_{ot}")
                ps = st["ps"]
                for k in range(2 * j, 2 * j + 2):
                    for nh in range(2):
                        nc.tensor.matmul(
                            ps[:, nh * 512:(nh + 1) * 512],
                            w1t[:, k, :, ot * P:(ot + 1) * P],
                            x_tiles[k][:, :, nh * 512:(nh + 1) * 512],
                            start=(k == 0), stop=(k == K2_1 - 1),
                            perf_mode=DR,
                        )
                if j == K2_1 // 2 - 1:
                    nc.scalar.activation(
                        feat[:, ot // 2, ot % 2], ps[:], AF.Relu,
                        scale=bn1[:, 0, ot:ot + 1], bias=bn1[:, 1, ot:ot + 1])

            def conv1_ot(i, ot, x_tiles, feat):
                st = {}
                for j in range(K2_1 // 2):
                    conv1_chunk(i, ot, x_tiles, feat, st, j)

            def conv1_fillers(i, ot, x_tiles, feat):
                st = {}
                return [
                    (lambda j=j: conv1_chunk(i, ot, x_tiles, feat, st, j))
                    for j in range(K2_1 // 2)
                ]

            def qk_ot(i, ot, feat, q_sb, k_sb):
                ps = ps_mm.tile([P, HW], F32, tag="mm", name=f"psqk_{i}_{ot}")
                for k in range(K2_2):
                    for nh in range(2):
                        nc.tensor.matmul(
                            ps[:, nh * 512:(nh + 1) * 512],
                            qkwt[:, k, :, ot * P:(ot + 1) * P],
                            feat[:, k, :, nh * 512:(nh + 1) * 512],
                            start=(k == 0), stop=(k == K2_2 - 1),
                            perf_mode=DR,
                        )
                if ot < HEADS:
                    nc.scalar.activation(q_sb[:, ot], ps[:], AF.Copy, scale=0.25)
                else:
                    nc.vector.tensor_tensor(k_sb[:, ot - HEADS], ps[:],
                                            embt[:], ALU.add)

            def v_pair(i, t, feat, v_sb):
                ps = ps_mm.tile([P, HW], F32, tag="mm", name=f"psv_{i}_{t}")
                for half in range(2):
                    yt = 2 * t + half
                    for k in range(K2_2):
                        nc.tensor.matmul(
                            ps[:, half * 512:(half + 1) * 512],
                            feat[:, k, :, yt * P:(yt + 1) * P],
                            vwt[:, k],
                            start=(k == 0), stop=(k == K2_2 - 1),
                            perf_mode=DR,
                        )
                nc.vector.tensor_copy(
                    v_sb[:, 2 * t:2 * t + 2],
                    ps[:].rearrange("p (t h d) -> p t h d", t=2, d=DV))

            # ---------- phase B (one head) ----------
            def head_logits(i, h, q_sb, k_sb, et, fillers=()):
                fillers = list(fillers)
                for yj in range(YT):
                    psl = ps_l.tile([P, HW], F32, tag="l", name=f"psl_{i}_{h}_{yj}")
                    for nh in range(2):
                        nc.tensor.matmul(
                            psl[:, nh * 512:(nh + 1) * 512],
                            k_sb[:, h, yj * P:(yj + 1) * P],
                            q_sb[:, h, nh * 512:(nh + 1) * 512],
                            start=True, stop=True,
                        )
                    nc.scalar.activation(et[:, yj], psl[:], AF.Exp,
                                         scale=1.0 / 128.0)
                    if yj % 2 == 1 and fillers:
                        fillers.pop(0)()
                for f in fillers:
                    f()

            def head_attn(i, h, et, v_sb, atf):
                dnb = ps_mm.tile([P, HW], F32, tag="mm", name=f"dnb_{i}_{h}")
                for nh in range(2):
                    for a in range(YT // 2):
                        nc.tensor.matmul(
                            dnb[:, nh * 512:(nh + 1) * 512],
                            ones8[:],
                            et[:, 2 * a:2 * a + 2, nh * 512:(nh + 1) * 512],
                            start=(a == 0), stop=(a == YT // 2 - 1),
                            perf_mode=DR,
                        )
                rc1 = rcp.tile([P, HW], F16, tag="rc", name=f"rc_{i}_{h}")
                nc.vector.tensor_scalar(rc1[:], dnb[:], RC_A, RC_B,
                                        ALU.mult, ALU.add)
                psa = ps_l.tile([P, HW], F32, tag="l", name=f"psa_{i}_{h}")
                for nh in range(2):
                    for a in range(YT // 2):
                        nc.tensor.matmul(
                            psa[:, nh * 512:(nh + 1) * 512],
                            v_sb[:, 2 * a:2 * a + 2, h],
                            et[:, 2 * a:2 * a + 2, nh * 512:(nh + 1) * 512],
                            start=(a == 0), stop=(a == YT // 2 - 1),
                            perf_mode=DR,
                        )
                nc.vector.scalar_tensor_tensor(atf[:, h], psa[:], 0.0, rc1[:],
                                               ALU.max, ALU.mult)

            # ---------- phase C (one out-tile) ----------
            def conv3_ot(i, ot, atf, xres_tiles, evict_dve, resid_pe,
                         alt_pool=False):
                ps = (ps_l.tile([P, HW], F32, tag="l", name=f"psc3_{i}_{ot}")
                      if alt_pool else
                      ps_mm.tile([P, HW], F32, tag="mm",
                                 name=f"psc3_{i}_{ot}"))
                for nh in range(2):
                    sl = slice(nh * 512, (nh + 1) * 512)
                    for k in range(K2_2):
                        nc.tensor.matmul(
                            ps[:, sl],
                            w3t[:, k, :, ot * P:(ot + 1) * P],
                            atf[:, 2 * k:2 * k + 2, sl],
                            start=(k == 0),
                            stop=(not resid_pe and k == K2_2 - 1),
                            perf_mode=DR,
                        )
                    if resid_pe:
                        nc.tensor.matmul(ps[:, sl], ident[:],
                                         xres_tiles[ot][:, sl],
                                         start=False, stop=True)
                if not resid_pe:
                    nc.vector.tensor_tensor(ps[:], ps[:], xres_tiles[ot][:],
                                            ALU.add)
                o_sb = outp.tile([P, HW], F16, tag="o", name=f"o_{i}_{ot}")
                if evict_dve:
                    nc.vector.tensor_scalar(o_sb[:], ps[:], add3[:, ot:ot + 1],
                                            0.0, ALU.add, ALU.max)
                else:
                    nc.scalar.activation(o_sb[:], ps[:], AF.Relu,
                                         bias=add3[:, ot:ot + 1])
                nc.scalar.dma_start(out_d.ap()[i, ot], o_sb[:])

            def prefetch_xres(i):
                tiles = []
                for ot in range(OT3):
                    xr = xresp.tile([P, HW], F16, tag="xr", name=f"xr_{i}_{ot}")
                    nc.sync.dma_start(xr[:], xres_d.ap()[i, ot])
                    tiles.append(xr)
                return tiles

            def alloc_bqv(i):
                feat = featp.tile([P, K2_2, 2, HW], F8, tag="feat",
                                  name=f"feat_{i}")
                q_sb = qkp.tile([P, HEADS, HW], F8, tag="q", name=f"q_{i}")
                k_sb = qkp.tile([P, HEADS, HW], F8, tag="k", name=f"k_{i}")
                v_sb = vp.tile([P, YT, HEADS, DV], F8, tag="v", name=f"v_{i}")
                return feat, q_sb, k_sb, v_sb

            # ================= emission =================
            # A(0)
            feat0, q0, k0, v0 = alloc_bqv(0)
            with nc.named_scope("A0"):
                xt0 = load_x(0)
                for ot in range(MID // P):
                    conv1_ot(0, ot, xt0, feat0)
                for ot in range(2 * HEADS):
                    qk_ot(0, ot, feat0, q0, k0)
                for t in range(YT // 2):
                    v_pair(0, t, feat0, v0)
            xt1 = load_x(1)
            xres0 = prefetch_xres(0)

            # B(0) interleaved with A(1)
            feat1, q1, k1, v1 = alloc_bqv(1)
            atf0 = atfp.tile([P, HEADS, HW], F8, tag="atf", name="atf_0")
            with nc.named_scope("B0_A1"):
                for h in range(HEADS):
                    et = etp.tile([P, YT, HW], F8, tag="et", name=f"et_0_{h}")
                    head_logits(0, h, q0, k0, et,
                                conv1_fillers(1, h, xt1, feat1))
                    head_attn(0, h, et, v0, atf0)
                for ot in range(2 * HEADS):
                    qk_ot(1, ot, feat1, q1, k1)
                for t in range(YT // 2):
                    v_pair(1, t, feat1, v1)
            xres1 = prefetch_xres(1)

            # C(0) interleaved with B(1)
            atf1 = atfp.tile([P, HEADS, HW], F8, tag="atf", name="atf_1")
            with nc.named_scope("C0_B1"):
                for h in range(HEADS):
                    et = etp.tile([P, YT, HW], F8, tag="et", name=f"et_1_{h}")
                    head_logits(1, h, q1, k1, et, [
                        (lambda ot=4 * h + j: conv3_ot(
                            0, ot, atf0, xres0, evict_dve=(ot % 2 == 1),
                            resid_pe=False))
                        for j in range(2)
                    ])
                    head_attn(1, h, et, v1, atf1)
                    for j in range(2, 4):
                        ot = 4 * h + j
                        conv3_ot(0, ot, atf0, xres0, evict_dve=(ot % 2 == 1),
                                 resid_pe=False)

            # C(1)
            with nc.named_scope("C1"):
                for ot in range(OT3):
                    conv3_ot(1, ot, atf1, xres1, evict_dve=(ot % 2 == 1),
                             resid_pe=True, alt_pool=(ot % 2 == 1))

    nc.compile()
    _BUILT["nc"] = nc
    return nc


def _prep_maps(x, conv1_w, gamma1, beta1, mean1, var1, qk_w, v_w, pos_h, pos_w,
               conv3_w, gamma3, beta3, mean3, var3):
    f16 = np.float16
    f8 = ml_dtypes.float8_e4m3
    inv1 = (gamma1 / np.sqrt(var1 + EPS)).astype(np.float32)
    add1 = (beta1 - mean1 * inv1).astype(np.float32)
    inv3 = (gamma3 / np.sqrt(var3 + EPS)).astype(np.float32)
    add3 = (beta3 - mean3 * inv3).astype(np.float32)

    def dr_weight(wt, k2):
        c, o = wt.shape
        return np.ascontiguousarray(
            wt.reshape(k2, 2, P, o).transpose(0, 2, 1, 3)).astype(f8)

    w1t = dr_weight(np.ascontiguousarray(conv1_w.T) * 64.0, K2_1)
    qk_mod = np.concatenate([qk_w[:HEADS * DQK] * (SCALE * 64.0),
                             qk_w[HEADS * DQK:] * 8.0], 0)
    qkwt = dr_weight(np.ascontiguousarray(qk_mod.T), K2_2)
    vwt = dr_weight(np.ascontiguousarray(v_w.T) * 8.0, K2_2)
    w3t = dr_weight(np.ascontiguousarray((conv3_w * inv3[:, None]).T) * 8.0,
                    K2_2)
    embt = np.ascontiguousarray(
        (pos_h[:, None, :] + pos_w[None, :, :]).reshape(HW, DQK).T * 8.0
    ).astype(np.float32)
    bn1 = np.stack([(inv1 / 64.0).reshape(MID // P, P).T,
                    add1.reshape(MID // P, P).T], 1)
    bn1 = np.ascontiguousarray(bn1).astype(np.float32)
    # out stored as 64x: bias folded as 64*add3
    add3_t = np.ascontiguousarray(
        (64.0 * add3).reshape(OT3, P).T).astype(np.float32)

    xr = x.reshape(N_IMG, CIN, HW)
    x8_all = np.ascontiguousarray(
        xr.reshape(N_IMG, K2_1, 2, P, HW).transpose(0, 1, 3, 2, 4)).astype(f8)
    xres_all = (xr.reshape(N_IMG, OT3, P, HW) * 64.0).astype(f16)

    in_maps = []
    for c in range(NCORES):
        sl = slice(c * IMGS, (c + 1) * IMGS)
        in_maps.append({
            "x8": np.ascontiguousarray(x8_all[sl]),
            "xres": np.ascontiguousarray(xres_all[sl]),
            "w1t": w1t, "qkwt": qkwt, "vwt": vwt, "w3t": w3t,
            "embt": embt, "bn1": bn1, "add3": add3_t,
        })
    return in_maps


def kernel(**inputs):
    nc = _build()
    inputs = {k: np.asarray(v) for k, v in inputs.items()}
    in_maps = _prep_maps(**inputs)
    res = run_bass_kernel_spmd(nc, in_maps, core_ids=list(range(NCORES)))
    out = np.concatenate([r["out"] for r in res.results], 0)
    return (out.reshape(N_IMG, CIN, H, W).astype(np.float32) / 64.0)
